# revision 36
# baseline (speedup 1.0000x reference)
"""MoE expert-routing kernel for Trainium2 (8 NeuronCores).

out[b] = x[b] @ weight[index[b]] + bias[index[b]]

Expert-parallel sharding (4 experts/core), host-side token routing
(stable argsort, capacity C per expert), fp16 operands/output with fp32
PSUM accumulation. Transposed compute layout — weights stationary,
tokens moving: out^T[o, t] = sum_i W[i, o] * xT[i, t] + b[o].

DMA layout: DRAM blocks are partition-major ([128, EPC*W]) so each DMA
descriptor covers multiple experts' bytes for one partition — fewer,
bigger packets. All bulk transfers ride the single SP HWDGE ring in
FIFO order (a second concurrent HWDGE ring does not add throughput and
starves; SWDGE bulk traffic degrades HWDGE). Input = chunk of 2
experts, then per-expert chunks so each expert's matmuls overlap the
next expert's stream tail; output = two 2-expert chunks. PE warm-up
dummy matmuls run during the input-DMA wait so the HAM clock gate
opens (1.2 -> 2.4 GHz) before the real matmuls. PSUM->SBUF bias-add
copies split across DVE (o-half 0) and ACT (o-half 1, with the
Identity-table load hoisted via a dummy activation).

Host-packed fp16 layout per core ([128, EPC*W], W = 512+4+2C):
  cols e*W.. : [w(k0,o0)|w(k0,o1)|w(k1,o0)|w(k1,o1)| b(f32 as 4 f16) |
                xT_h0 | xT_h1]  for expert e
Output [128, EPC, 2, C] fp16 (partition-major), untransposed on host.
"""

import numpy as np

B, E, DIN, DOUT = 4096, 32, 256, 256
NCORES = 8
EPC = E // NCORES

WARM_MMS = 6          # dummy matmuls to open the HAM clock gate
WARM_N = 512          # free dim of each dummy matmul

TRACE = False
LAST_RESULT = None

_PROGRAM_CACHE = {}


def _build_program(C):
    import concourse.bass as bass
    import concourse.mybir as mybir
    import concourse.tile as tile
    from concourse import bacc

    f32 = mybir.dt.float32
    f16 = mybir.dt.float16

    W = -(-(4 * 128 + 4 + 2 * C) // 16) * 16   # 16-aligned for xbar tiles
    boff = 4 * 128            # 4 f16 cols = 2 f32 bias cols (bit-packed)
    xoff = boff + 4
    CK = 512                  # token chunk per PSUM group (f32 bank limit)

    nc = bacc.Bacc("TRN2", target_bir_lowering=False, debug=False,
                   enable_asserts=False)

    # DRAM holds the transpose of the SBUF image: fully contiguous for
    # the xbar DMA-transpose path (4KB M2S tiles, no per-partition descs)
    blk_d = nc.dram_tensor("blk", [EPC * W, 128], f16, kind="ExternalInput")
    out_d = nc.dram_tensor("out", [128, EPC, 2, C], f16,
                           kind="ExternalOutput")

    with tile.TileContext(nc) as tc:
        with (
            tc.tile_pool(name="bin", bufs=1) as bpool,
            tc.tile_pool(name="oout", bufs=1) as opool,
            tc.tile_pool(name="wrm", bufs=1) as wpool,
            tc.tile_pool(name="psum", bufs=6, space=bass.MemorySpace.PSUM)
                as ppool,
            tc.tile_pool(name="dpsum", bufs=1, space=bass.MemorySpace.PSUM)
                as dppool,
        ):
            # dummy Identity activation on a const AP: hoists ACT's
            # ~1.3us table load to the queue head, before any DMA wait
            dact = wpool.tile([128, 1], f32, tag="dact")
            nc.scalar.add(dact[:], nc.const_aps.aps[(f32, 0.0)], 1.0)

            # ---- input: all on the SP ring (FIFO; a second concurrent
            # HWDGE ring does not add throughput and starves). First chunk
            # covers 2 experts; the rest go per-expert so each expert's
            # matmuls overlap the next expert's stream tail and the
            # per-DMA semaphore straggle.
            bt = bpool.tile([128, EPC * W], f16, tag="bt")
            nc.sync.dma_start_transpose(bt[:, 0:2 * W],
                                        blk_d.ap()[0:2 * W, :])
            nc.sync.dma_start_transpose(bt[:, 2 * W:3 * W],
                                        blk_d.ap()[2 * W:3 * W, :])
            nc.sync.dma_start_transpose(bt[:, 3 * W:4 * W],
                                        blk_d.ap()[3 * W:4 * W, :])

            # ---- PE warm-up: dummy matmuls on a memset tile ----
            dum = wpool.tile([128, WARM_N], f16, tag="dum")
            nc.vector.memset(dum[:], 0.0)
            dps = dppool.tile([128, WARM_N], f32, tag="dps")
            for i in range(WARM_MMS):
                nc.tensor.matmul(dps[:], dum[:, 0:128], dum[:],
                                 start=True, stop=True)
            dsb = wpool.tile([128, 1], f32, tag="dsb")
            nc.vector.tensor_copy(dsb[:], dps[:, 0:1])

            ot = opool.tile([128, EPC, 2, C], f16, tag="ot")
            for e in range(EPC):
                eb = e * W
                for oh in range(2):
                    for ck in range(0, C, CK):
                        cw = min(CK, C - ck)
                        ps = ppool.tile([128, CK], f32)
                        for k in range(2):
                            nc.tensor.matmul(
                                ps[:, :cw],
                                bt[:, eb + (k * 2 + oh) * 128:
                                    eb + (k * 2 + oh + 1) * 128],
                                bt[:, eb + xoff + k * C + ck:
                                    eb + xoff + k * C + ck + cw],
                                start=(k == 0), stop=(k == 1),
                            )
                        if oh == 0:
                            nc.vector.tensor_scalar_add(
                                ot[:, e, 0, ck:ck + cw], ps[:, :cw],
                                bt[:, eb + boff:eb + boff + 2].bitcast(f32))
                        else:
                            nc.scalar.add(
                                ot[:, e, 1, ck:ck + cw], ps[:, :cw],
                                bt[:, eb + boff + 2:eb + boff + 4]
                                .bitcast(f32))
                if e == 1:
                    # first output chunk as soon as e0+e1 are done
                    nc.sync.dma_start(out_d.ap()[:, 0:2], ot[:, 0:2])
            nc.sync.dma_start(out_d.ap()[:, 2:4], ot[:, 2:4])

    nc.compile()
    return nc


def _route(index):
    order = np.argsort(index, kind="stable")
    counts = np.bincount(index, minlength=E)
    offs = np.zeros(E + 1, np.int64)
    offs[1:] = np.cumsum(counts)
    C = max(64, int(-(-int(counts.max()) // 16) * 16))
    return order, counts, offs, C


def _pack_core(x16, w16, b32, order, offs, C, c):
    W = -(-(4 * 128 + 4 + 2 * C) // 16) * 16
    boff = 4 * 128
    xoff = boff + 4
    blk = np.zeros((EPC, 128, W), np.float16)
    for sl in range(EPC):
        e = c * EPC + sl
        toks = order[offs[e]:offs[e + 1]]
        xT = x16[toks].T
        for k in range(2):
            for oh in range(2):
                blk[sl, :, (k * 2 + oh) * 128:(k * 2 + oh + 1) * 128] = \
                    w16[e, k * 128:(k + 1) * 128, oh * 128:(oh + 1) * 128]
        # f32 bias bit-packed into f16 column pairs
        bv = b32[e].view(np.float16).reshape(256, 2)
        blk[sl, :, boff:boff + 2] = bv[0:128]
        blk[sl, :, boff + 2:boff + 4] = bv[128:256]
        blk[sl, :, xoff:xoff + xT.shape[1]] = xT[0:128]
        blk[sl, :, xoff + C:xoff + C + xT.shape[1]] = xT[128:256]
    # transposed for the xbar DMA path: [EPC*W, 128]
    return np.ascontiguousarray(
        blk.transpose(1, 0, 2).reshape(128, EPC * W).T)


def kernel(x, index, weight, bias):
    from concourse.bass_utils import run_bass_kernel_spmd

    global LAST_RESULT

    x = np.asarray(x, np.float32)
    index = np.asarray(index, np.int32)
    weight = np.asarray(weight, np.float32)
    bias = np.asarray(bias, np.float32)

    order, counts, offs, C = _route(index)

    if C not in _PROGRAM_CACHE:
        _PROGRAM_CACHE[C] = _build_program(C)
    nc = _PROGRAM_CACHE[C]

    x16 = x.astype(np.float16)
    w16 = weight.astype(np.float16)
    b32 = np.ascontiguousarray(bias, np.float32)
    in_maps = []
    for c in range(NCORES):
        in_maps.append({
            "blk": _pack_core(x16, w16, b32, order, offs, C, c),
        })

    kwargs = {}
    if TRACE:
        kwargs = dict(trace=True, trace_cores=list(range(NCORES)))
    res = run_bass_kernel_spmd(nc, in_maps, core_ids=list(range(NCORES)),
                               **kwargs)
    LAST_RESULT = res

    out = np.empty((B, DOUT), np.float32)
    for c in range(NCORES):
        oc = res.results[c]["out"]  # [128, EPC, 2, C] fp16
        for sl in range(EPC):
            e = c * EPC + sl
            toks = order[offs[e]:offs[e + 1]]
            oe = oc[:, sl].transpose(2, 1, 0).reshape(C, DOUT)
            out[toks] = oe[:len(toks)].astype(np.float32)
    return out


# revision 38
# speedup vs baseline: 1.0859x; 1.0859x over previous
"""MoE expert-routing kernel for Trainium2 (8 NeuronCores).

out[b] = x[b] @ weight[index[b]] + bias[index[b]]

Expert-parallel sharding (4 experts/core), host-side token routing
(stable argsort, capacity C per expert), fp16 operands/output with fp32
PSUM accumulation. Transposed compute layout — weights stationary,
tokens moving: out^T[o, t] = sum_i W[i, o] * xT[i, t] + b[o].

DMA layout: DRAM blocks are partition-major ([128, EPC*W]) so each DMA
descriptor covers multiple experts' bytes for one partition — fewer,
bigger packets. All bulk transfers ride the single SP HWDGE ring in
FIFO order (a second concurrent HWDGE ring does not add throughput and
starves; SWDGE bulk traffic degrades HWDGE). Input = chunk of 2
experts, then per-expert chunks so each expert's matmuls overlap the
next expert's stream tail; output = two 2-expert chunks. PE warm-up
dummy matmuls run during the input-DMA wait so the HAM clock gate
opens (1.2 -> 2.4 GHz) before the real matmuls. PSUM->SBUF bias-add
copies split across DVE (o-half 0) and ACT (o-half 1, with the
Identity-table load hoisted via a dummy activation).

Host-packed fp16 layout per core ([128, EPC*W], W = 512+4+2C):
  cols e*W.. : [w(k0,o0)|w(k0,o1)|w(k1,o0)|w(k1,o1)| b(f32 as 4 f16) |
                xT_h0 | xT_h1]  for expert e
Output [128, EPC, 2, C] fp16 (partition-major), untransposed on host.
"""

import numpy as np

B, E, DIN, DOUT = 4096, 32, 256, 256
NCORES = 8
EPC = E // NCORES

WARM_MMS = 6          # dummy matmuls to open the HAM clock gate
WARM_N = 512          # free dim of each dummy matmul

TRACE = False
LAST_RESULT = None

_PROGRAM_CACHE = {}


def _build_program(C):
    import concourse.bass as bass
    import concourse.mybir as mybir
    import concourse.tile as tile
    from concourse import bacc

    f32 = mybir.dt.float32
    f16 = mybir.dt.float16

    W = 4 * 128 + 4 + 2 * C
    boff = 4 * 128            # 4 f16 cols = 2 f32 bias cols (bit-packed)
    xoff = boff + 4
    CK = 512                  # token chunk per PSUM group (f32 bank limit)

    nc = bacc.Bacc("TRN2", target_bir_lowering=False, debug=False,
                   enable_asserts=False)

    blk_d = nc.dram_tensor("blk", [128, EPC * W], f16, kind="ExternalInput")
    out_d = nc.dram_tensor("out", [128, EPC, 2, C], f16,
                           kind="ExternalOutput")

    with tile.TileContext(nc) as tc:
        with (
            tc.tile_pool(name="bin", bufs=1) as bpool,
            tc.tile_pool(name="oout", bufs=1) as opool,
            tc.tile_pool(name="wrm", bufs=1) as wpool,
            tc.tile_pool(name="psum", bufs=6, space=bass.MemorySpace.PSUM)
                as ppool,
            tc.tile_pool(name="dpsum", bufs=1, space=bass.MemorySpace.PSUM)
                as dppool,
        ):
            # dummy Identity activation on a const AP: hoists ACT's
            # ~1.3us table load to the queue head, before any DMA wait
            dact = wpool.tile([128, 1], f32, tag="dact")
            nc.scalar.add(dact[:], nc.const_aps.aps[(f32, 0.0)], 1.0)

            # ---- input: all on the SP ring (FIFO; a second concurrent
            # HWDGE ring does not add throughput and starves). First chunk
            # covers 2 experts; the rest go per-expert so each expert's
            # matmuls overlap the next expert's stream tail and the
            # per-DMA semaphore straggle.
            bt = bpool.tile([128, EPC * W], f16, tag="bt")
            nc.sync.dma_start(bt[:, 0:2 * W], blk_d.ap()[:, 0:2 * W])
            nc.sync.dma_start(bt[:, 2 * W:3 * W], blk_d.ap()[:, 2 * W:3 * W])
            nc.sync.dma_start(bt[:, 3 * W:4 * W], blk_d.ap()[:, 3 * W:4 * W])

            # ---- PE warm-up: dummy matmuls on a memset tile ----
            dum = wpool.tile([128, WARM_N], f16, tag="dum")
            nc.vector.memset(dum[:], 0.0)
            dps = dppool.tile([128, WARM_N], f32, tag="dps")
            for i in range(WARM_MMS):
                nc.tensor.matmul(dps[:], dum[:, 0:128], dum[:],
                                 start=True, stop=True)
            dsb = wpool.tile([128, 1], f32, tag="dsb")
            nc.vector.tensor_copy(dsb[:], dps[:, 0:1])

            ot = opool.tile([128, EPC, 2, C], f16, tag="ot")
            for e in range(EPC):
                eb = e * W
                for oh in range(2):
                    for ck in range(0, C, CK):
                        cw = min(CK, C - ck)
                        ps = ppool.tile([128, CK], f32)
                        for k in range(2):
                            nc.tensor.matmul(
                                ps[:, :cw],
                                bt[:, eb + (k * 2 + oh) * 128:
                                    eb + (k * 2 + oh + 1) * 128],
                                bt[:, eb + xoff + k * C + ck:
                                    eb + xoff + k * C + ck + cw],
                                start=(k == 0), stop=(k == 1),
                            )
                        if oh == 0:
                            nc.vector.tensor_scalar_add(
                                ot[:, e, 0, ck:ck + cw], ps[:, :cw],
                                bt[:, eb + boff:eb + boff + 2].bitcast(f32))
                        else:
                            nc.scalar.add(
                                ot[:, e, 1, ck:ck + cw], ps[:, :cw],
                                bt[:, eb + boff + 2:eb + boff + 4]
                                .bitcast(f32))
                if e == 1:
                    # first output chunk as soon as e0+e1 are done
                    nc.sync.dma_start(out_d.ap()[:, 0:2], ot[:, 0:2])
            # final chunk issued by ACT right behind its own last copy
            # (engine-local FIFO: no cross-engine wait before the issue)
            nc.scalar.dma_start(out_d.ap()[:, 2:4], ot[:, 2:4])

    nc.compile()
    return nc


def _route(index):
    order = np.argsort(index, kind="stable")
    counts = np.bincount(index, minlength=E)
    offs = np.zeros(E + 1, np.int64)
    offs[1:] = np.cumsum(counts)
    C = max(64, int(-(-int(counts.max()) // 16) * 16))
    return order, counts, offs, C


def _pack_core(x16, w16, b32, order, offs, C, c):
    W = 4 * 128 + 4 + 2 * C
    boff = 4 * 128
    xoff = boff + 4
    blk = np.zeros((EPC, 128, W), np.float16)
    for sl in range(EPC):
        e = c * EPC + sl
        toks = order[offs[e]:offs[e + 1]]
        xT = x16[toks].T
        for k in range(2):
            for oh in range(2):
                blk[sl, :, (k * 2 + oh) * 128:(k * 2 + oh + 1) * 128] = \
                    w16[e, k * 128:(k + 1) * 128, oh * 128:(oh + 1) * 128]
        # f32 bias bit-packed into f16 column pairs
        bv = b32[e].view(np.float16).reshape(256, 2)
        blk[sl, :, boff:boff + 2] = bv[0:128]
        blk[sl, :, boff + 2:boff + 4] = bv[128:256]
        blk[sl, :, xoff:xoff + xT.shape[1]] = xT[0:128]
        blk[sl, :, xoff + C:xoff + C + xT.shape[1]] = xT[128:256]
    # partition-major: [128, EPC*W]
    return np.ascontiguousarray(blk.transpose(1, 0, 2).reshape(128, EPC * W))


def kernel(x, index, weight, bias):
    from concourse.bass_utils import run_bass_kernel_spmd

    global LAST_RESULT

    x = np.asarray(x, np.float32)
    index = np.asarray(index, np.int32)
    weight = np.asarray(weight, np.float32)
    bias = np.asarray(bias, np.float32)

    order, counts, offs, C = _route(index)

    if C not in _PROGRAM_CACHE:
        _PROGRAM_CACHE[C] = _build_program(C)
    nc = _PROGRAM_CACHE[C]

    x16 = x.astype(np.float16)
    w16 = weight.astype(np.float16)
    b32 = np.ascontiguousarray(bias, np.float32)
    in_maps = []
    for c in range(NCORES):
        in_maps.append({
            "blk": _pack_core(x16, w16, b32, order, offs, C, c),
        })

    kwargs = {}
    if TRACE:
        kwargs = dict(trace=True, trace_cores=list(range(NCORES)))
    res = run_bass_kernel_spmd(nc, in_maps, core_ids=list(range(NCORES)),
                               **kwargs)
    LAST_RESULT = res

    out = np.empty((B, DOUT), np.float32)
    for c in range(NCORES):
        oc = res.results[c]["out"]  # [128, EPC, 2, C] fp16
        for sl in range(EPC):
            e = c * EPC + sl
            toks = order[offs[e]:offs[e + 1]]
            oe = oc[:, sl].transpose(2, 1, 0).reshape(C, DOUT)
            out[toks] = oe[:len(toks)].astype(np.float32)
    return out
